# revision 38
# baseline (speedup 1.0000x reference)
"""DeepSeekMoE kernel for 8 TRN2 NeuronCores.

Sharding: expert-parallel. Core e owns expert e's FFN (W1[e], W2[e]) and a
1/8 H-shard of the shared expert (tensor-parallel). The tiny gate
(sigmoid + top-2 over E=8) runs on host; tokens are gathered per expert,
padded to a common cap, and shipped pre-transposed so every device-side
matmul contracts over the partition dimension. Each core returns
  ye: [N_CAP, D]  routed-expert outputs, already scaled by the combine weight
  sh: [T, D]      shared-expert partial (its H-shard's contribution)
Host scatters ye back by token index and sums the 8 sh partials (the output
gather performs the MoE combine; no on-device collectives needed).

Compute dtype: bf16 operands, fp32 PSUM accumulation (rel err ~3e-3).
"""

import hashlib
import sys

sys.path.insert(0, "/opt/trn_rl_repo")

import numpy as np
import ml_dtypes

import concourse.bass as bass
import concourse.bacc as bacc
import concourse.mybir as mybir
import concourse.tile as tile
from concourse.bass_utils import run_bass_kernel_spmd

BF16 = ml_dtypes.bfloat16
F32 = np.float32

T, D, E, TOP_K, H = 2048, 1024, 8, 2, 4096
HS = H // 8          # shared-expert hidden shard per core
KD = D // 128        # 8  k-chunks over D
KH = H // 128        # 32 k-chunks over H
KHS = HS // 128      # 4  k-chunks over the shared shard
N_CORES = 8

_DT = mybir.dt.bfloat16
_cache: dict = {}
_wcache: dict = {}


def _build(n_cap: int):
    """Build + finalize the SPMD device program for token cap n_cap."""
    assert n_cap % 64 == 0
    nt = -(-n_cap // 128)                   # 128-token chunks (last may be 64)
    tchunks = []                            # (start, size) 512-sized chunks
    s = 0
    while s < n_cap:
        tchunks.append((s, min(512, n_cap - s)))
        s += 512

    nc = bacc.Bacc("TRN2", target_bir_lowering=False, debug=False)

    xe_d = nc.dram_tensor("xe", [128, KD, n_cap], _DT, kind="ExternalInput")
    xt_d = nc.dram_tensor("xt", [4, 128, KD, 512], _DT, kind="ExternalInput")
    w1_d = nc.dram_tensor("w1", [KH // 2, 128, KD, 256], _DT, kind="ExternalInput")
    w2_d = nc.dram_tensor("w2", [8, 128, KH, 128], _DT, kind="ExternalInput")
    ws1_d = nc.dram_tensor("ws1", [128, KD, HS], _DT, kind="ExternalInput")
    ws2_d = nc.dram_tensor("ws2", [128, KHS, D], _DT, kind="ExternalInput")
    b1_d = nc.dram_tensor("b1c", [128, KH], mybir.dt.float32, kind="ExternalInput")
    bs1_d = nc.dram_tensor("bs1c", [128, KHS], mybir.dt.float32, kind="ExternalInput")
    # combine weights replicated across partitions: [128, n_cap]
    wrow_d = nc.dram_tensor("wrow", [128, n_cap], mybir.dt.float32, kind="ExternalInput")

    ye_d = nc.dram_tensor("ye", [D, n_cap], mybir.dt.float32, kind="ExternalOutput")
    sh_d = nc.dram_tensor("sh", [T, D], mybir.dt.float32, kind="ExternalOutput")

    gelu = mybir.ActivationFunctionType.Gelu

    with tile.TileContext(nc) as tc:
        with (
            tc.tile_pool(name="resident", bufs=1) as rpool,
            tc.tile_pool(name="w1s", bufs=4) as w1pool,
            tc.tile_pool(name="w2s", bufs=3) as w2pool,
            tc.tile_pool(name="xts", bufs=2) as xtpool,
            tc.tile_pool(name="psum", bufs=4, space="PSUM") as pspool,
            tc.tile_pool(name="outs", bufs=2) as opool,
        ):
            # ---- PE warmup: ~4us of dummy matmuls while the real input DMAs
            # are in flight, so the HAM clock gate is at 2.4GHz when phase B
            # starts (cold PE runs at 1.2GHz for its first ~3.4us of work).
            scratch = rpool.tile([128, 512], _DT)
            nc.vector.memset(scratch[:], 0.0)
            wps = pspool.tile([128, 512], mybir.dt.float32)
            for _ in range(9):
                nc.tensor.matmul(wps[:], scratch[:, 0:128], scratch[:], start=True, stop=True)
            # ---- phase-B-critical loads first: they gate the first matmul.
            # Queues are FIFO, so issue the first half-slab and xe halves in
            # the order the first PSUM group consumes them.
            w1s0 = w1pool.tile([128, KD, 256], _DT)
            nc.sync.dma_start(w1s0[:, :, 0:128], w1_d[0, :, :, 0:128])
            xe_sb = rpool.tile([128, KD, n_cap], _DT)
            nc.sync.dma_start(xe_sb[:, 0:KD // 2, :], xe_d[:, 0:KD // 2, :])
            nc.sync.dma_start(xe_sb[:, KD // 2:, :], xe_d[:, KD // 2:, :])
            nc.sync.dma_start(w1s0[:, :, 128:256], w1_d[0, :, :, 128:256])
            b1_sb = rpool.tile([128, KH], mybir.dt.float32)
            nc.sync.dma_start(b1_sb[:], b1_d[:])

            hT = rpool.tile([128, KH, n_cap], _DT)    # gelu(x@W1^T), H on partitions
            hsT = rpool.tile([128, KHS, T], _DT)      # shared-expert hidden

            # Shared-expert weights are needed only from phase D/E onward.
            # Emitting their loads at t=0 starves the critical phase-B loads
            # (all DGE paths share the 16 SDMA engines), so they are issued
            # during phase C, in usage order.
            ws1_sb = rpool.tile([128, KD, HS], _DT)
            ws2_sb = rpool.tile([128, KHS, D], _DT)

            # ---- phase B: routed GEMM1  hT[h,:] = gelu(W1T_h^T contract XeT) ----
            # w1 is streamed as 16 pair-slabs (2 h-chunks each) to keep the
            # sync sequencer's ~0.6us-per-DMA issue cost off the critical path.
            for hp in range(KH // 2):
                if hp == 0:
                    w1s = w1s0
                else:
                    w1s = w1pool.tile([128, KD, 256], _DT)
                    nc.sync.dma_start(w1s[:], w1_d[hp])
                for hh in range(2):
                    h = 2 * hp + hh
                    for (t0, tsz) in tchunks:
                        ps = pspool.tile([128, 512], mybir.dt.float32)
                        for k in range(KD):
                            nc.tensor.matmul(
                                ps[:, :tsz],
                                w1s[:, k, hh * 128:hh * 128 + 128],
                                xe_sb[:, k, t0:t0 + tsz],
                                start=(k == 0),
                                stop=(k == KD - 1),
                            )
                        nc.scalar.activation(
                            hT[:, h, t0:t0 + tsz], ps[:, :tsz], gelu,
                            bias=b1_sb[:, h:h + 1],
                        )

            # ---- phase C: routed GEMM2 (tokens moving) + combine-weight scale ----
            wrow_sb = rpool.tile([128, n_cap], mybir.dt.float32)
            nc.sync.dma_start(wrow_sb[:], wrow_d[:])
            bs1_sb = rpool.tile([128, KHS], mybir.dt.float32)
            nc.sync.dma_start(bs1_sb[:], bs1_d[:])

            for d in range(8):
                w2s = w2pool.tile([128, KH, 128], _DT)
                nc.sync.dma_start(w2s[:], w2_d[d])
                # shared-expert weights trickle behind the first two slabs
                if d == 0:
                    nc.sync.dma_start(ws1_sb[:], ws1_d[:])
                elif d == 1:
                    nc.sync.dma_start(ws2_sb[:], ws2_d[:])
                for (t0, tsz) in tchunks:
                    ps = pspool.tile([128, 512], mybir.dt.float32)
                    for k in range(KH):
                        nc.tensor.matmul(
                            ps[:, :tsz],
                            w2s[:, k, :],
                            hT[:, k, t0:t0 + tsz],
                            start=(k == 0),
                            stop=(k == KH - 1),
                        )
                    eo = opool.tile([128, 512], mybir.dt.float32, tag="eo")
                    nc.vector.tensor_mul(
                        eo[:, :tsz], ps[:, :tsz], wrow_sb[:, t0:t0 + tsz]
                    )
                    nc.sync.dma_start(
                        ye_d[d * 128:(d + 1) * 128, t0:t0 + tsz], eo[:, :tsz]
                    )

            # ---- phase D: shared GEMM1 over all T tokens ----
            for tcn in range(4):
                xts = xtpool.tile([128, KD, 512], _DT)
                nc.sync.dma_start(xts[:], xt_d[tcn])
                for hs in range(KHS):
                    ps = pspool.tile([128, 512], mybir.dt.float32)
                    for k in range(KD):
                        nc.tensor.matmul(
                            ps[:],
                            ws1_sb[:, k, hs * 128:(hs + 1) * 128],
                            xts[:, k, :],
                            start=(k == 0),
                            stop=(k == KD - 1),
                        )
                    nc.scalar.activation(
                        hsT[:, hs, tcn * 512:(tcn + 1) * 512], ps[:], gelu,
                        bias=bs1_sb[:, hs:hs + 1],
                    )

            # ---- phase E: shared GEMM2 ----
            for t in range(T // 128):
                for dh in range(2):
                    ps = pspool.tile([128, 512], mybir.dt.float32)
                    for k in range(KHS):
                        nc.tensor.matmul(
                            ps[:],
                            hsT[:, k, t * 128:(t + 1) * 128],
                            ws2_sb[:, k, dh * 512:(dh + 1) * 512],
                            start=(k == 0),
                            stop=(k == KHS - 1),
                        )
                    so = opool.tile([128, 512], mybir.dt.float32, tag="so")
                    nc.vector.tensor_copy(so[:], ps[:])
                    nc.sync.dma_start(
                        sh_d[t * 128:(t + 1) * 128, dh * 512:(dh + 1) * 512], so[:]
                    )

    nc.finalize()
    return nc


def _routing(xf, Wg, bg, bias):
    """Host gate: fp64 for a stable top-2 ranking (matches fp32 reference
    ordering except for ~1e-7-wide ties, which don't occur at these margins)."""
    logits = xf.astype(np.float64) @ Wg.T.astype(np.float64) + bg + bias
    scores = (1.0 / (1.0 + np.exp(-logits))).astype(np.float32)
    # stable sort => ties break toward the lower expert index, like lax.top_k
    top_idx = np.argsort(-scores, axis=1, kind="stable")[:, :TOP_K]
    top_w = np.take_along_axis(scores, top_idx, axis=1)
    return top_idx, top_w


def kernel(x, Wg, bg, bias, W1, b1, W2, b2, Ws1, bs1, Ws2, bs2):
    x = np.asarray(x, F32)
    Wg, bg, bias = np.asarray(Wg, F32), np.asarray(bg, F32), np.asarray(bias, F32)
    W1, b1 = np.asarray(W1, F32), np.asarray(b1, F32)
    W2, b2 = np.asarray(W2, F32), np.asarray(b2, F32)
    Ws1, bs1 = np.asarray(Ws1, F32), np.asarray(bs1, F32)
    Ws2, bs2 = np.asarray(Ws2, F32), np.asarray(bs2, F32)

    xf = x.reshape(-1, D)
    top_idx, top_w = _routing(xf, Wg, bg, bias)

    sels, ws = [], []
    for e in range(E):
        pick = (top_idx == e)
        sel = np.where(pick.any(axis=1))[0]
        w = np.where(pick[sel, 0], top_w[sel, 0], top_w[sel, 1]).astype(F32)
        sels.append(sel)
        ws.append(w)
    n_cap = max(128, -(-max(len(s) for s in sels) // 64) * 64)

    if n_cap not in _cache:
        _cache[n_cap] = _build(n_cap)
    nc = _cache[n_cap]

    x_bf = xf.astype(BF16)
    # xt: [4, 128, KD, 512]  (token-chunk major, partition-major inside)
    xt = np.ascontiguousarray(
        x_bf.T.reshape(KD, 128, 4, 512).transpose(2, 1, 0, 3)
    )

    # Per-expert weight re-layouts are input-independent; cache across calls
    # (keyed by content hash, so a reused buffer can't serve stale layouts).
    hsh = hashlib.blake2b(digest_size=16)
    for a in (W1, W2, Ws1, Ws2, b1, bs1):
        hsh.update(np.ascontiguousarray(a).data)
    wkey = hsh.hexdigest()
    wmaps = _wcache.get(wkey)
    if wmaps is None:
        wmaps = []
        for e in range(E):
            hs0 = e * HS
            wmaps.append({
                # W1[e]: [H, D] -> W1T [D, H] -> [KH//2, 128, KD, 256] pair-slabs
                "w1": np.ascontiguousarray(
                    W1[e].T.reshape(KD, 128, KH // 2, 256)
                    .transpose(2, 1, 0, 3).astype(BF16)
                ),
                # W2[e]: [D, H] -> W2T [H, D] -> [8, 128, KH, 128] d-slabs
                "w2": np.ascontiguousarray(
                    W2[e].T.reshape(KH, 128, 8, 128).transpose(2, 1, 0, 3).astype(BF16)
                ),
                # Ws1 shard rows -> Ws1T [D, HS] -> [128, KD, HS]
                "ws1": np.ascontiguousarray(
                    Ws1[hs0:hs0 + HS].T.reshape(KD, 128, HS)
                    .transpose(1, 0, 2).astype(BF16)
                ),
                # Ws2 shard cols -> Ws2T [HS, D] -> [128, KHS, D]
                "ws2": np.ascontiguousarray(
                    Ws2[:, hs0:hs0 + HS].T.reshape(KHS, 128, D)
                    .transpose(1, 0, 2).astype(BF16)
                ),
                "b1c": np.ascontiguousarray(b1[e].reshape(KH, 128).T),
                "bs1c": np.ascontiguousarray(bs1[hs0:hs0 + HS].reshape(KHS, 128).T),
            })
        _wcache.clear()
        _wcache[wkey] = wmaps

    in_maps = []
    for e in range(E):
        sel, w = sels[e], ws[e]
        xe = np.zeros((n_cap, D), BF16)
        xe[: len(sel)] = x_bf[sel]
        # [128, KD, n_cap]
        xe_t = np.ascontiguousarray(xe.T.reshape(KD, 128, n_cap).transpose(1, 0, 2))
        wpad = np.zeros(n_cap, F32)
        wpad[: len(w)] = w
        in_maps.append({
            "xe": xe_t,
            "xt": xt,
            "wrow": np.ascontiguousarray(np.broadcast_to(wpad, (128, n_cap))),
            **wmaps[e],
        })

    res = run_bass_kernel_spmd(nc, in_maps, core_ids=list(range(N_CORES)))

    out = np.zeros((T, D), F32)
    for c in range(N_CORES):
        out += res.results[c]["sh"]
    for e in range(E):
        sel = sels[e]
        out[sel] += res.results[e]["ye"][:, : len(sel)].T
    # biases handled host-side: per-token weighted b2, plus bs2
    wdense = np.zeros((T, E), F32)
    np.put_along_axis(wdense, top_idx, top_w, axis=1)
    out += wdense @ b2
    out += bs2
    return out.reshape(x.shape)


# revision 42
# speedup vs baseline: 1.1526x; 1.1526x over previous
"""DeepSeekMoE kernel for 8 TRN2 NeuronCores.

Sharding: expert-parallel. Core e owns expert e's FFN (W1[e], W2[e]) and a
1/8 H-shard of the shared expert (tensor-parallel). The tiny gate
(sigmoid + top-2 over E=8) runs on host; tokens are gathered per expert,
padded to a common cap, and shipped pre-transposed so every device-side
matmul contracts over the partition dimension. Each core returns
  ye: [N_CAP, D]  routed-expert outputs, already scaled by the combine weight
  sh: [T, D]      shared-expert partial (its H-shard's contribution)
Host scatters ye back by token index and sums the 8 sh partials (the output
gather performs the MoE combine; no on-device collectives needed).

Compute dtype: bf16 operands, fp32 PSUM accumulation (rel err ~3e-3).
"""

import hashlib
import sys

sys.path.insert(0, "/opt/trn_rl_repo")

import numpy as np
import ml_dtypes

import concourse.bass as bass
import concourse.bacc as bacc
import concourse.mybir as mybir
import concourse.tile as tile
from concourse.bass_utils import run_bass_kernel_spmd

BF16 = ml_dtypes.bfloat16
F32 = np.float32

T, D, E, TOP_K, H = 2048, 1024, 8, 2, 4096
HS = H // 8          # shared-expert hidden shard per core
KD = D // 128        # 8  k-chunks over D
KH = H // 128        # 32 k-chunks over H
KHS = HS // 128      # 4  k-chunks over the shared shard
N_CORES = 8

_DT = mybir.dt.bfloat16
_cache: dict = {}
_wcache: dict = {}


def _build(n_cap: int):
    """Build + finalize the SPMD device program for token cap n_cap."""
    assert n_cap % 64 == 0
    nt = -(-n_cap // 128)                   # 128-token chunks (last may be 64)
    tchunks = []                            # (start, size) 512-sized chunks
    s = 0
    while s < n_cap:
        tchunks.append((s, min(512, n_cap - s)))
        s += 512

    nc = bacc.Bacc("TRN2", target_bir_lowering=False, debug=False)

    xe_d = nc.dram_tensor("xe", [128, KD, n_cap], _DT, kind="ExternalInput")
    xt_d = nc.dram_tensor("xt", [4, 128, KD, 512], _DT, kind="ExternalInput")
    w1_d = nc.dram_tensor("w1", [KH // 2, 128, KD, 256], _DT, kind="ExternalInput")
    w2_d = nc.dram_tensor("w2", [8, 128, KH, 128], _DT, kind="ExternalInput")
    ws1_d = nc.dram_tensor("ws1", [128, KD, HS], _DT, kind="ExternalInput")
    ws2_d = nc.dram_tensor("ws2", [128, KHS, D], _DT, kind="ExternalInput")
    b1_d = nc.dram_tensor("b1c", [128, KH], mybir.dt.float32, kind="ExternalInput")
    bs1_d = nc.dram_tensor("bs1c", [128, KHS], mybir.dt.float32, kind="ExternalInput")
    # combine weights replicated across partitions: [128, n_cap]
    wrow_d = nc.dram_tensor("wrow", [128, n_cap], mybir.dt.float32, kind="ExternalInput")

    ye_d = nc.dram_tensor("ye", [D, n_cap], mybir.dt.float32, kind="ExternalOutput")
    # shared-expert partial leaves in bf16: halves the phase-E output DMA,
    # which otherwise caps the kernel tail (8 partials summed in f32 on host)
    sh_d = nc.dram_tensor("sh", [T, D], _DT, kind="ExternalOutput")

    gelu = mybir.ActivationFunctionType.Gelu

    with tile.TileContext(nc) as tc:
        with (
            tc.tile_pool(name="resident", bufs=1) as rpool,
            tc.tile_pool(name="w1s", bufs=4) as w1pool,
            tc.tile_pool(name="w2s", bufs=3) as w2pool,
            tc.tile_pool(name="xts", bufs=2) as xtpool,
            tc.tile_pool(name="psum", bufs=4, space="PSUM") as pspool,
            tc.tile_pool(name="outs", bufs=4) as opool,
        ):
            # ---- PE warmup: ~4us of dummy matmuls while the real input DMAs
            # are in flight, so the HAM clock gate is at 2.4GHz when phase B
            # starts (cold PE runs at 1.2GHz for its first ~3.4us of work).
            scratch = rpool.tile([128, 512], _DT)
            nc.vector.memset(scratch[:], 0.0)
            wps = pspool.tile([128, 512], mybir.dt.float32)
            for _ in range(9):
                nc.tensor.matmul(wps[:], scratch[:, 0:128], scratch[:], start=True, stop=True)
            # ---- phase-B-critical loads first: they gate the first matmul.
            # Queues are FIFO, so issue the first half-slab and xe halves in
            # the order the first PSUM group consumes them.
            w1s0 = w1pool.tile([128, KD, 256], _DT)
            nc.sync.dma_start(w1s0[:, :, 0:128], w1_d[0, :, :, 0:128])
            xe_sb = rpool.tile([128, KD, n_cap], _DT)
            nc.sync.dma_start(xe_sb[:, 0:KD // 2, :], xe_d[:, 0:KD // 2, :])
            nc.sync.dma_start(xe_sb[:, KD // 2:, :], xe_d[:, KD // 2:, :])
            nc.sync.dma_start(w1s0[:, :, 128:256], w1_d[0, :, :, 128:256])
            b1_sb = rpool.tile([128, KH], mybir.dt.float32)
            nc.sync.dma_start(b1_sb[:], b1_d[:])

            hT = rpool.tile([128, KH, n_cap], _DT)    # gelu(x@W1^T), H on partitions
            hsT = rpool.tile([128, KHS, T], _DT)      # shared-expert hidden

            # Shared-expert weights are needed only from phase D/E onward.
            # Emitting their loads at t=0 starves the critical phase-B loads
            # (all DGE paths share the 16 SDMA engines), so they are issued
            # during phase C, in usage order.
            ws1_sb = rpool.tile([128, KD, HS], _DT)
            ws2_sb = rpool.tile([128, KHS, D], _DT)

            # ---- phase B: routed GEMM1  hT[h,:] = gelu(W1T_h^T contract XeT) ----
            # w1 is streamed as 16 pair-slabs (2 h-chunks each) to keep the
            # sync sequencer's ~0.6us-per-DMA issue cost off the critical path.
            for hp in range(KH // 2):
                if hp == 0:
                    w1s = w1s0
                else:
                    w1s = w1pool.tile([128, KD, 256], _DT)
                    nc.sync.dma_start(w1s[:], w1_d[hp])
                for hh in range(2):
                    h = 2 * hp + hh
                    for (t0, tsz) in tchunks:
                        ps = pspool.tile([128, 512], mybir.dt.float32)
                        for k in range(KD):
                            nc.tensor.matmul(
                                ps[:, :tsz],
                                w1s[:, k, hh * 128:hh * 128 + 128],
                                xe_sb[:, k, t0:t0 + tsz],
                                start=(k == 0),
                                stop=(k == KD - 1),
                            )
                        nc.scalar.activation(
                            hT[:, h, t0:t0 + tsz], ps[:, :tsz], gelu,
                            bias=b1_sb[:, h:h + 1],
                        )

            # ---- phase C: routed GEMM2 (tokens moving) + combine-weight scale ----
            wrow_sb = rpool.tile([128, n_cap], mybir.dt.float32)
            nc.sync.dma_start(wrow_sb[:], wrow_d[:])
            bs1_sb = rpool.tile([128, KHS], mybir.dt.float32)
            nc.sync.dma_start(bs1_sb[:], bs1_d[:])

            for d in range(8):
                w2s = w2pool.tile([128, KH, 128], _DT)
                nc.sync.dma_start(w2s[:], w2_d[d])
                # shared-expert weights trickle behind the first two slabs
                if d == 0:
                    nc.sync.dma_start(ws1_sb[:], ws1_d[:])
                elif d == 1:
                    nc.sync.dma_start(ws2_sb[:], ws2_d[:])
                for (t0, tsz) in tchunks:
                    ps = pspool.tile([128, 512], mybir.dt.float32)
                    for k in range(KH):
                        nc.tensor.matmul(
                            ps[:, :tsz],
                            w2s[:, k, :],
                            hT[:, k, t0:t0 + tsz],
                            start=(k == 0),
                            stop=(k == KH - 1),
                        )
                    eo = opool.tile([128, 512], mybir.dt.float32, tag="eo")
                    nc.vector.tensor_mul(
                        eo[:, :tsz], ps[:, :tsz], wrow_sb[:, t0:t0 + tsz]
                    )
                    nc.sync.dma_start(
                        ye_d[d * 128:(d + 1) * 128, t0:t0 + tsz], eo[:, :tsz]
                    )

            # ---- phase D: shared GEMM1 over all T tokens ----
            for tcn in range(4):
                xts = xtpool.tile([128, KD, 512], _DT)
                nc.sync.dma_start(xts[:], xt_d[tcn])
                for hs in range(KHS):
                    ps = pspool.tile([128, 512], mybir.dt.float32)
                    for k in range(KD):
                        nc.tensor.matmul(
                            ps[:],
                            ws1_sb[:, k, hs * 128:(hs + 1) * 128],
                            xts[:, k, :],
                            start=(k == 0),
                            stop=(k == KD - 1),
                        )
                    nc.scalar.activation(
                        hsT[:, hs, tcn * 512:(tcn + 1) * 512], ps[:], gelu,
                        bias=bs1_sb[:, hs:hs + 1],
                    )

            # ---- phase E: shared GEMM2 ----
            for t in range(T // 128):
                for dh in range(2):
                    ps = pspool.tile([128, 512], mybir.dt.float32)
                    for k in range(KHS):
                        nc.tensor.matmul(
                            ps[:],
                            hsT[:, k, t * 128:(t + 1) * 128],
                            ws2_sb[:, k, dh * 512:(dh + 1) * 512],
                            start=(k == 0),
                            stop=(k == KHS - 1),
                        )
                    so = opool.tile([128, 512], _DT, tag="so")
                    nc.vector.tensor_copy(so[:], ps[:])
                    nc.sync.dma_start(
                        sh_d[t * 128:(t + 1) * 128, dh * 512:(dh + 1) * 512], so[:]
                    )

    nc.finalize()
    return nc


def _routing(xf, Wg, bg, bias):
    """Host gate: fp64 for a stable top-2 ranking (matches fp32 reference
    ordering except for ~1e-7-wide ties, which don't occur at these margins)."""
    logits = xf.astype(np.float64) @ Wg.T.astype(np.float64) + bg + bias
    scores = (1.0 / (1.0 + np.exp(-logits))).astype(np.float32)
    # stable sort => ties break toward the lower expert index, like lax.top_k
    top_idx = np.argsort(-scores, axis=1, kind="stable")[:, :TOP_K]
    top_w = np.take_along_axis(scores, top_idx, axis=1)
    return top_idx, top_w


def kernel(x, Wg, bg, bias, W1, b1, W2, b2, Ws1, bs1, Ws2, bs2):
    x = np.asarray(x, F32)
    Wg, bg, bias = np.asarray(Wg, F32), np.asarray(bg, F32), np.asarray(bias, F32)
    W1, b1 = np.asarray(W1, F32), np.asarray(b1, F32)
    W2, b2 = np.asarray(W2, F32), np.asarray(b2, F32)
    Ws1, bs1 = np.asarray(Ws1, F32), np.asarray(bs1, F32)
    Ws2, bs2 = np.asarray(Ws2, F32), np.asarray(bs2, F32)

    xf = x.reshape(-1, D)
    top_idx, top_w = _routing(xf, Wg, bg, bias)

    sels, ws = [], []
    for e in range(E):
        pick = (top_idx == e)
        sel = np.where(pick.any(axis=1))[0]
        w = np.where(pick[sel, 0], top_w[sel, 0], top_w[sel, 1]).astype(F32)
        sels.append(sel)
        ws.append(w)
    n_cap = max(128, -(-max(len(s) for s in sels) // 64) * 64)

    if n_cap not in _cache:
        _cache[n_cap] = _build(n_cap)
    nc = _cache[n_cap]

    x_bf = xf.astype(BF16)
    # xt: [4, 128, KD, 512]  (token-chunk major, partition-major inside)
    xt = np.ascontiguousarray(
        x_bf.T.reshape(KD, 128, 4, 512).transpose(2, 1, 0, 3)
    )

    # Per-expert weight re-layouts are input-independent; cache across calls
    # (keyed by content hash, so a reused buffer can't serve stale layouts).
    hsh = hashlib.blake2b(digest_size=16)
    for a in (W1, W2, Ws1, Ws2, b1, bs1):
        hsh.update(np.ascontiguousarray(a).data)
    wkey = hsh.hexdigest()
    wmaps = _wcache.get(wkey)
    if wmaps is None:
        wmaps = []
        for e in range(E):
            hs0 = e * HS
            wmaps.append({
                # W1[e]: [H, D] -> W1T [D, H] -> [KH//2, 128, KD, 256] pair-slabs
                "w1": np.ascontiguousarray(
                    W1[e].T.reshape(KD, 128, KH // 2, 256)
                    .transpose(2, 1, 0, 3).astype(BF16)
                ),
                # W2[e]: [D, H] -> W2T [H, D] -> [8, 128, KH, 128] d-slabs
                "w2": np.ascontiguousarray(
                    W2[e].T.reshape(KH, 128, 8, 128).transpose(2, 1, 0, 3).astype(BF16)
                ),
                # Ws1 shard rows -> Ws1T [D, HS] -> [128, KD, HS]
                "ws1": np.ascontiguousarray(
                    Ws1[hs0:hs0 + HS].T.reshape(KD, 128, HS)
                    .transpose(1, 0, 2).astype(BF16)
                ),
                # Ws2 shard cols -> Ws2T [HS, D] -> [128, KHS, D]
                "ws2": np.ascontiguousarray(
                    Ws2[:, hs0:hs0 + HS].T.reshape(KHS, 128, D)
                    .transpose(1, 0, 2).astype(BF16)
                ),
                "b1c": np.ascontiguousarray(b1[e].reshape(KH, 128).T),
                "bs1c": np.ascontiguousarray(bs1[hs0:hs0 + HS].reshape(KHS, 128).T),
            })
        _wcache.clear()
        _wcache[wkey] = wmaps

    in_maps = []
    for e in range(E):
        sel, w = sels[e], ws[e]
        xe = np.zeros((n_cap, D), BF16)
        xe[: len(sel)] = x_bf[sel]
        # [128, KD, n_cap]
        xe_t = np.ascontiguousarray(xe.T.reshape(KD, 128, n_cap).transpose(1, 0, 2))
        wpad = np.zeros(n_cap, F32)
        wpad[: len(w)] = w
        in_maps.append({
            "xe": xe_t,
            "xt": xt,
            "wrow": np.ascontiguousarray(np.broadcast_to(wpad, (128, n_cap))),
            **wmaps[e],
        })

    res = run_bass_kernel_spmd(nc, in_maps, core_ids=list(range(N_CORES)))

    out = np.zeros((T, D), F32)
    for c in range(N_CORES):
        out += res.results[c]["sh"].astype(F32)
    for e in range(E):
        sel = sels[e]
        out[sel] += res.results[e]["ye"][:, : len(sel)].T
    # biases handled host-side: per-token weighted b2, plus bs2
    wdense = np.zeros((T, E), F32)
    np.put_along_axis(wdense, top_idx, top_w, axis=1)
    out += wdense @ b2
    out += bs2
    return out.reshape(x.shape)
